# revision 1
# baseline (speedup 1.0000x reference)
"""Mimi-style GQA attention (RoPE + 250-wide sliding causal window) on 8 TRN2 NeuronCores.

Sharding (per spec hint): core c handles batch b=c//4 and KV-head group g=c%4
(4 query heads + 1 KV head). Wq/Wk/Wv column-sharded, Wo row-sharded along the
head dim; each core emits a partial [S, D] output; host sums the 4 partials
per batch.

Per-core pipeline (all matmul inputs fp16, PSUM accumulation fp32):
  1. qT/kT/vT projections from host-pre-transposed hsT (contraction dim on
     partitions), RoPE applied as qT*cos + (PERM @ qT)*sin_signed where PERM is
     a rotate-half permutation done on the tensor engine.
  2. Banded flash attention in scoresT [k, q] orientation: per 128-wide k-tile
     only the 384-wide in-window q-range is computed; exp (with fused 1/8
     scale) on ScalarE; band mask multiply on VectorE; PV accumulated per
     512-wide q-strip into [65, 512] PSUM tiles where row 64 (a ones column in
     the V stationary) accumulates the softmax denominators for free.
  3. Row-sum reciprocal + partition-broadcast + multiply normalizes, then the
     row-sharded Wo projection produces the partial output.
"""
import os
import sys

for _p in ("/opt/trn_rl_repo", "/root/.axon_site/_ro/trn_rl_repo"):
    if os.path.isdir(_p) and _p not in sys.path:
        sys.path.append(_p)

import numpy as np
import concourse.bass as bass
import concourse.mybir as mybir
import concourse.tile as tile
from concourse import bacc
from concourse.bass_utils import run_bass_kernel_spmd
from concourse.masks import make_identity

F32 = mybir.dt.float32
F16 = mybir.dt.float16
AF = mybir.ActivationFunctionType
OP = mybir.AluOpType

B, S, D = 2, 2048, 1024
H, HK, HD = 16, 4, 64
WINDOW = 250
SCALE = 1.0 / np.sqrt(HD)
THETA = 10000.0
NKT = S // 128          # 16 k-tiles
NST = S // 512          # 4 q-strips
WIN = 384               # padded per-k-tile q-window


def _pv_pieces(s):
    """PV pieces for q-strip s: list of (kt, c0, c1) window-column ranges.

    The PSUM bank is pre-seeded by a zero matmul with start=True (PSUM
    has_written group-clear is bank-wide, so interleaved start flags are
    unusable); every piece then accumulates with start=False.
    """
    out = []
    for kt in range(max(0, 4 * s - 2), min(NKT - 1, 4 * s + 3) + 1):
        j0 = 128 * kt
        w = min(WIN, S - j0)
        c_lo = max(0, 512 * s - j0)
        c_hi = min(w, 512 * (s + 1) - j0)
        if c_lo < c_hi:
            out.append((kt, c_lo, c_hi))
    return out


def _build(debug=False):
    nc = bacc.Bacc(None, target_bir_lowering=False)
    if debug:
        dbg_qT0 = nc.declare_dram_parameter("dbg_qT0", [128, S], F16, isOutput=True)
        dbg_kdup = nc.declare_dram_parameter("dbg_kdup", [128, S], F16, isOutput=True)
        dbg_vT = nc.declare_dram_parameter("dbg_vT", [64, S], F16, isOutput=True)
        dbg_et = nc.declare_dram_parameter("dbg_et", [128, WIN], F16, isOutput=True)
        dbg_pv = nc.declare_dram_parameter("dbg_pv", [65, 512], F32, isOutput=True)
        dbg_aT0 = nc.declare_dram_parameter("dbg_aT0", [128, S], F16, isOutput=True)

    hsT = nc.declare_dram_parameter("hsT", [8, 128, S], F16, isOutput=False)
    wq = nc.declare_dram_parameter("wqT", [8, 128, 256], F16, isOutput=False)
    wk = nc.declare_dram_parameter("wkT", [8, 128, 64], F16, isOutput=False)
    wv = nc.declare_dram_parameter("wvT", [8, 128, 64], F16, isOutput=False)
    wo = nc.declare_dram_parameter("woT", [2, 128, D], F16, isOutput=False)
    cosd = nc.declare_dram_parameter("cos2", [128, S], F16, isOutput=False)
    sind = nc.declare_dram_parameter("sinS2", [128, S], F16, isOutput=False)
    maskd = nc.declare_dram_parameter("bandmask", [128, WIN], F16, isOutput=False)
    permd = nc.declare_dram_parameter("permT", [128, 128], F16, isOutput=False)
    o_part = nc.declare_dram_parameter("o_part", [S, D], F32, isOutput=True)

    with tile.TileContext(nc) as tc:
        with (
            tc.tile_pool(name="persist", bufs=1) as pp,
            tc.tile_pool(name="work", bufs=3) as wk_pool,
            tc.tile_pool(name="expm", bufs=26) as ep,
            tc.tile_pool(name="norm", bufs=2) as npool,
            tc.tile_pool(name="ost", bufs=4) as opool,
        ):
            # ---- persistent loads: small weights first (gpsimd queue), hsT after ----
            wq_sb = pp.tile([128, 8, 256], F16, tag="wq")
            wk_sb = pp.tile([128, 8, 64], F16, tag="wk")
            wv_sb = pp.tile([128, 8, 64], F16, tag="wv")
            nc.sync.dma_start(out=wq_sb, in_=wq.rearrange("a p c -> p a c"))
            nc.sync.dma_start(out=wk_sb, in_=wk.rearrange("a p c -> p a c"))
            nc.sync.dma_start(out=wv_sb, in_=wv.rearrange("a p c -> p a c"))
            wo_sb = pp.tile([128, 2, D], F16, tag="wo")
            nc.gpsimd.dma_start(out=wo_sb, in_=wo.rearrange("a p c -> p a c"))
            cos_sb = pp.tile([128, S], F16, tag="cos")
            sin_sb = pp.tile([128, S], F16, tag="sin")
            nc.gpsimd.dma_start(out=cos_sb, in_=cosd[:, :])
            nc.gpsimd.dma_start(out=sin_sb, in_=sind[:, :])
            mask_sb = pp.tile([128, WIN], F16, tag="mask")
            nc.gpsimd.dma_start(out=mask_sb, in_=maskd[:, :])
            perm_sb = pp.tile([128, 128], F16, tag="perm")
            nc.gpsimd.dma_start(out=perm_sb, in_=permd[:, :])
            ht = []
            for dt in range(8):
                t = pp.tile([128, S], F16, tag=f"ht{dt}")
                nc.sync.dma_start(out=t[:, 0:1024], in_=hsT[dt][:, 0:1024])
                nc.sync.dma_start(out=t[:, 1024:2048], in_=hsT[dt][:, 1024:2048])
                ht.append(t)
            ident = pp.tile([64, 64], F16, tag="ident")
            make_identity(nc, ident)
            zero65 = pp.tile([128, 65], F16, tag="zero65")
            nc.gpsimd.memset(zero65, 0.0)
            warm = pp.tile([128, 512], F16, tag="warm")
            nc.gpsimd.memset(warm, 0.0)

            qT = [pp.tile([128, S], F16, tag=f"qT{m}", name=f"qT{m}") for m in range(2)]
            kdup = pp.tile([128, S], F16, tag="kdup")
            vT = pp.tile([64, S], F16, tag="vT")
            vaug = [pp.tile([128, 65], F16, tag=f"vaug{kt}", name=f"vaug{kt}") for kt in range(NKT)]
            aT = [pp.tile([128, S], F16, tag=f"aT{m}", name=f"aT{m}") for m in range(2)]
            stag = [pp.tile([64, S], F16, tag=f"stag{m}", name=f"stag{m}") for m in range(2)]

            # ---- phase 1: projections + rope ----
            with (
                tc.tile_pool(name="ps1", bufs=4, space="PSUM") as ps1,
                tc.tile_pool(name="ps1b", bufs=2, space="PSUM") as ps1b,
            ):
                # HAM warm-up: ~5us of back-to-back matmuls while DMAs stream
                wps = ps1b.tile([128, 512], F32, tag="rot", name="warmps")
                for _w in range(24):
                    nc.tensor.matmul(wps, warm[:, 0:128], warm,
                                     start=True, stop=True)
                def rope(dest_ap, raw_ps, rows, strip, tagp):
                    """dest = raw*cos + (PERM @ raw)*sin_signed on [rows, 512]."""
                    sl = bass.ts(strip, 512)
                    raw = wk_pool.tile([rows, 512], F16, tag=f"raw{tagp}")
                    nc.scalar.copy(raw, raw_ps)
                    rot = ps1b.tile([rows, 512], F32, tag="rot")
                    nc.tensor.matmul(rot, perm_sb[0:rows, 0:rows], raw,
                                     start=True, stop=True)
                    t1 = wk_pool.tile([rows, 512], F16, tag=f"t1{tagp}")
                    nc.vector.tensor_tensor(out=t1, in0=rot, in1=sin_sb[0:rows, sl],
                                            op=OP.mult)
                    t2 = wk_pool.tile([rows, 512], F16, tag=f"t2{tagp}")
                    nc.gpsimd.tensor_tensor(out=t2, in0=raw, in1=cos_sb[0:rows, sl],
                                            op=OP.mult)
                    nc.vector.tensor_tensor(out=dest_ap, in0=t1, in1=t2, op=OP.add)

                def proj_group(lhs_of_dt, rows, consume):
                    """One projection over all 4 strips, dt-outer so each
                    stationary is loaded once and reused across strips."""
                    pstiles = [ps1.tile([rows, 512], F32, tag="proj",
                                        name=f"pj{rows}_{_s}") for _s in range(NST)]
                    for dt in range(8):
                        for strip in range(NST):
                            nc.tensor.matmul(pstiles[strip], lhs_of_dt(dt),
                                             ht[dt][:, bass.ts(strip, 512)],
                                             start=(dt == 0), stop=(dt == 7))
                    for strip in range(NST):
                        consume(pstiles[strip], strip)

                for m in range(2):
                    proj_group(lambda dt: wq_sb[:, dt, bass.ts(m, 128)], 128,
                               lambda ps, strip: rope(qT[m][:, bass.ts(strip, 512)],
                                                      ps, 128, strip, "q"))

                def k_consume(ps, strip):
                    sl = bass.ts(strip, 512)
                    rope(kdup[0:64, sl], ps, 64, strip, "k")
                    nc.sync.dma_start(out=kdup[64:128, sl], in_=kdup[0:64, sl])
                proj_group(lambda dt: wk_sb[:, dt], 64, k_consume)
                proj_group(lambda dt: wv_sb[:, dt], 64,
                           lambda ps, strip: nc.scalar.copy(vT[:, bass.ts(strip, 512)], ps))

                for kt in range(NKT):
                    pvt = ps1b.tile([128, 64], F16, tag="vtr")
                    nc.tensor.transpose(pvt, vT[:, bass.ts(kt, 128)], ident)
                    nc.vector.tensor_copy(vaug[kt][:, 0:64], pvt)
                    nc.vector.memset(vaug[kt][:, 64:65], 1.0)

            # ---- phases 2+3: banded attention + output projection ----
            with (
                tc.tile_pool(name="ps2", bufs=2, space="PSUM") as ps2,
                tc.tile_pool(name="pspv", bufs=1, space="PSUM") as pspv,
                tc.tile_pool(name="ps3", bufs=2, space="PSUM") as ps3,
            ):
                expm = {}
                for s in range(NST):
                    # fresh expT tiles for this strip's new k-tiles
                    kts = range(max(0, 4 * s - 2), min(NKT - 1, 4 * s + 3) + 1)
                    for kt in kts:
                        if (kt, 0) in expm:
                            continue
                        j0 = 128 * kt
                        w = min(WIN, S - j0)
                        for h in range(4):
                            m, half = h // 2, (h % 2) * 64
                            pss = ps2.tile([128, WIN], F32, tag="sc")
                            nc.tensor.matmul(
                                pss[:, 0:w],
                                kdup[half:half + 64, bass.ts(kt, 128)],
                                qT[m][half:half + 64, j0:j0 + w],
                                start=True, stop=True)
                            et = ep.tile([128, WIN], F16, tag="e")
                            nc.scalar.activation(et[:, 0:w], pss[:, 0:w], AF.Exp,
                                                 scale=float(SCALE))
                            if w == WIN:
                                blk = bass.AP(tensor=et.tensor, offset=et.offset,
                                              ap=[list(et.ap[0]), [248, 2], [1, 136]])
                                mblk = bass.AP(tensor=mask_sb.tensor, offset=mask_sb.offset,
                                               ap=[list(mask_sb.ap[0]), [248, 2], [1, 136]])
                                nc.vector.tensor_tensor(out=blk, in0=blk, in1=mblk,
                                                        op=OP.mult)
                            else:
                                nc.vector.tensor_tensor(out=et[:, 0:w], in0=et[:, 0:w],
                                                        in1=mask_sb[:, 0:w], op=OP.mult)
                            expm[(kt, h)] = et

                    pieces = _pv_pieces(s)
                    for hp in range(2):
                        pvp = pspv.tile([65, 2, 512], F32, tag=f"pv{hp}",
                                        name=f"pv{hp}_{s}")
                        for hh in range(2):
                            nc.tensor.matmul(pvp[:, hh, :], zero65, ht[0][:, 0:512],
                                             start=True, stop=False)
                        for i, (kt, c0, c1) in enumerate(pieces):
                            base = 128 * kt + c0 - 512 * s
                            last = i == len(pieces) - 1
                            for hh in range(2):
                                nc.tensor.matmul(pvp[:, hh, base:base + (c1 - c0)],
                                                 vaug[kt],
                                                 expm[(kt, 2 * hp + hh)][:, c0:c1],
                                                 start=False, stop=last)
                        if debug and s == 1 and hp == 0:
                            dbgpv_sb = npool.tile([65, 512], F32, tag="dbgpv")
                            nc.scalar.copy(dbgpv_sb, pvp[:, 0, :])
                            nc.sync.dma_start(out=dbg_pv[:, :], in_=dbgpv_sb)
                        # softmax denominators, lane-parallel reciprocal
                        srow = npool.tile([65, 1024], F32, tag="srow")
                        nc.scalar.copy(srow[64:65, :], pvp[64:65, :, :])
                        rsp = npool.tile([8, 128], F32, tag="rsp")
                        nc.sync.dma_start(out=rsp, in_=srow[64:65, :])
                        rc16 = npool.tile([8, 128], F16, tag="rc16")
                        nc.vector.reciprocal(rsp, rsp)
                        nc.vector.tensor_copy(rc16, rsp)
                        r0 = npool.tile([1, 1024], F16, tag="r0")
                        nc.sync.dma_start(out=r0, in_=rc16)
                        bc = npool.tile([64, 1024], F16, tag="bc")
                        nc.gpsimd.partition_broadcast(bc, r0)
                        nc.vector.tensor_tensor(
                            out=aT[hp][0:64, bass.ts(s, 512)],
                            in0=pvp[0:64, 0, :], in1=bc[:, 0:512], op=OP.mult)
                        nc.vector.tensor_tensor(
                            out=stag[hp][0:64, bass.ts(s, 512)],
                            in0=pvp[0:64, 1, :], in1=bc[:, 512:1024], op=OP.mult)
                        nc.sync.dma_start(out=aT[hp][64:128, bass.ts(s, 512)],
                                          in_=stag[hp][0:64, bass.ts(s, 512)])
                    for st in range(4 * s, 4 * s + 4):
                        psos = [ps3.tile([128, 512], F32, tag="pso", name=f"pso{st}_{_d}")
                                for _d in range(2)]
                        for ch in range(2):
                            for dstrip in range(2):
                                nc.tensor.matmul(psos[dstrip], aT[ch][:, bass.ts(st, 128)],
                                                 wo_sb[:, ch, bass.ts(dstrip, 512)],
                                                 start=(ch == 0), stop=(ch == 1))
                        ost = opool.tile([128, 1024], F32, tag="o")
                        nc.scalar.copy(ost[:, 0:512], psos[0])
                        nc.vector.tensor_copy(ost[:, 512:1024], psos[1])
                        nc.sync.dma_start(out=o_part[bass.ts(st, 128), :], in_=ost)
                if debug:
                    nc.sync.dma_start(out=dbg_qT0[:, :], in_=qT[0])
                    nc.sync.dma_start(out=dbg_kdup[:, :], in_=kdup)
                    nc.sync.dma_start(out=dbg_vT[:, :], in_=vT)
                    nc.sync.dma_start(out=dbg_et[:, :], in_=expm[(5, 0)])
                    nc.sync.dma_start(out=dbg_aT0[:, :], in_=aT[0])


    nc.compile()
    return nc


_NC = {}


def _get_nc(debug=False):
    if debug not in _NC:
        _NC[debug] = _build(debug)
    return _NC[debug]


def _host_inputs(hidden_states, position_ids, Wq, Wk, Wv, Wo):
    hs = np.asarray(hidden_states, np.float32)
    Wq = np.asarray(Wq, np.float32)
    Wk = np.asarray(Wk, np.float32)
    Wv = np.asarray(Wv, np.float32)
    Wo = np.asarray(Wo, np.float32)

    hsT16 = [np.ascontiguousarray(hs[b].T).astype(np.float16).reshape(8, 128, S)
             for b in range(B)]

    inv_freq = (1.0 / (THETA ** (np.arange(0, HD, 2, dtype=np.float32) / HD))).astype(np.float32)
    cos2, sin2 = [], []
    for b in range(B):
        pos = np.asarray(position_ids[b]).astype(np.float32)
        freqs = pos[:, None] * inv_freq[None, :]          # [S, 32]
        cosf = np.cos(freqs).T                            # [32, S]
        sinf = np.sin(freqs).T
        cos64 = np.concatenate([cosf, cosf], axis=0)      # [64, S]
        sin64s = np.concatenate([-sinf, sinf], axis=0)    # sign-folded
        cos2.append(np.concatenate([cos64, cos64], axis=0).astype(np.float16))
        sin2.append(np.concatenate([sin64s, sin64s], axis=0).astype(np.float16))

    p = np.arange(128)[:, None]
    c = np.arange(WIN)[None, :]
    bandmask = ((p <= c) & (c < p + WINDOW)).astype(np.float16)

    perm = np.zeros((64, 64), np.float32)
    for i in range(32):
        perm[i, i + 32] = 1.0
        perm[i + 32, i] = 1.0
    perm2 = np.kron(np.eye(2, dtype=np.float32), perm)    # [128, 128]
    permT = np.ascontiguousarray(perm2.T).astype(np.float16)

    in_maps = []
    for core in range(8):
        b, g = divmod(core, 4)
        wqT = np.ascontiguousarray(Wq[256 * g:256 * (g + 1)].T).astype(np.float16).reshape(8, 128, 256)
        wkT = np.ascontiguousarray(Wk[64 * g:64 * (g + 1)].T).astype(np.float16).reshape(8, 128, 64)
        wvT = np.ascontiguousarray(Wv[64 * g:64 * (g + 1)].T).astype(np.float16).reshape(8, 128, 64)
        woT = np.ascontiguousarray(Wo[:, 256 * g:256 * (g + 1)].T).astype(np.float16).reshape(2, 128, D)
        in_maps.append({
            "hsT": hsT16[b], "wqT": wqT, "wkT": wkT, "wvT": wvT, "woT": woT,
            "cos2": cos2[b], "sinS2": sin2[b],
            "bandmask": bandmask, "permT": permT,
        })
    return in_maps


def run_spmd(hidden_states, attention_mask, position_ids, Wq, Wk, Wv, Wo, debug=False, **spmd_kwargs):
    nc = _get_nc(debug)
    in_maps = _host_inputs(hidden_states, position_ids, Wq, Wk, Wv, Wo)
    res = run_bass_kernel_spmd(nc, in_maps, list(range(8)), **spmd_kwargs)
    out = np.zeros((B, S, D), np.float32)
    for core in range(8):
        out[core // 4] += np.asarray(res.results[core]["o_part"], np.float32)
    return out, res


def kernel(hidden_states, attention_mask, position_ids, Wq, Wk, Wv, Wo):
    out, _ = run_spmd(hidden_states, attention_mask, position_ids, Wq, Wk, Wv, Wo)
    return out



# revision 4
# speedup vs baseline: 1.2139x; 1.2139x over previous
"""Mimi-style GQA attention (RoPE + 250-wide sliding causal window) on 8 TRN2 NeuronCores.

Sharding (per spec hint): core c handles batch b=c//4 and KV-head group g=c%4
(4 query heads + 1 KV head). Wq/Wk/Wv column-sharded, Wo row-sharded along the
head dim; each core emits a partial [S, D] output (fp16); host sums the 4
partials per batch in fp32.

v2 design notes (vs the 174-200us baseline):
  * The baseline's softmax-normalization chain (1-lane copies, 3 SB-SB DMAs,
    gpsimd partition_broadcast per head pair) created >3.4us PE-idle windows,
    oscillating the HAM clock gate between 2.4 and 1.2 GHz for ~half the span.
  * PV stationaries are [V|ones] for even heads and [ones|V] for odd heads, so
    the softmax denominator lands broadcast across 64 PSUM partitions for free
    (matmul cost depends only on moving columns). Normalization is then two
    lane-aligned reciprocals, two small partition-shift DMAs, and two
    lane-aligned multiplies writing the head-stacked aT directly.
  * K+V projections fused into one [Wv|Wk] stationary (kT upper / vT lower
    partitions); RoPE for K runs entirely on partitions 64-127.
  * Projections stream dt-tiles of hsT as the DMA delivers them; 8 warm-up
    matmuls cover only the initial DMA latency.
  * PV accumulations use bank-aligned [128,2,512] PSUM tiles with start=True on
    the first banded piece (no zero-seed matmuls).
  * One shared 8-bank PSUM ring serves scores, PV, and out-projection tiles;
    out-projection matmuls of strip s-1 interleave between strip s's score
    bursts so the PE never waits on the normalization chain.
  * Output is written fp16 (halves output DMA bytes; host accumulates fp32).
"""
import os
import sys

for _p in ("/opt/trn_rl_repo", "/root/.axon_site/_ro/trn_rl_repo"):
    if os.path.isdir(_p) and _p not in sys.path:
        sys.path.append(_p)

import numpy as np
import concourse.bass as bass
import concourse.mybir as mybir
import concourse.tile as tile
from concourse import bacc
from concourse.bass_utils import run_bass_kernel_spmd
from concourse.masks import make_identity

F32 = mybir.dt.float32
F16 = mybir.dt.float16
AF = mybir.ActivationFunctionType
OP = mybir.AluOpType

B, S, D = 2, 2048, 1024
H, HK, HD = 16, 4, 64
WINDOW = 250
SCALE = 1.0 / np.sqrt(HD)
THETA = 10000.0
NKT = S // 128          # 16 k-tiles
NST = S // 512          # 4 q-strips
WIN = 384               # padded per-k-tile q-window

MASK_BATCH = True       # one strided mask multiply per head pair
PV_SEED = False         # fall back to zero-seed matmuls if bank clear misbehaves


def _pv_pieces(s):
    """PV pieces for q-strip s: list of (kt, c0, c1) window-column ranges."""
    out = []
    for kt in range(max(0, 4 * s - 2), min(NKT - 1, 4 * s + 3) + 1):
        j0 = 128 * kt
        w = min(WIN, S - j0)
        c_lo = max(0, 512 * s - j0)
        c_hi = min(w, 512 * (s + 1) - j0)
        if c_lo < c_hi:
            out.append((kt, c_lo, c_hi))
    return out


def _build(debug=False):
    nc = bacc.Bacc(None, target_bir_lowering=False)
    if debug:
        dbg_qT0 = nc.declare_dram_parameter("dbg_qT0", [128, S], F16, isOutput=True)
        dbg_kdup = nc.declare_dram_parameter("dbg_kdup", [128, S], F16, isOutput=True)
        dbg_et = nc.declare_dram_parameter("dbg_et", [128, 2, WIN], F16, isOutput=True)
        dbg_aT0 = nc.declare_dram_parameter("dbg_aT0", [128, S], F16, isOutput=True)

    hsT = nc.declare_dram_parameter("hsT", [8, 128, S], F16, isOutput=False)
    wq = nc.declare_dram_parameter("wqT", [8, 128, 256], F16, isOutput=False)
    wkv = nc.declare_dram_parameter("wkvT", [8, 128, 128], F16, isOutput=False)
    wo = nc.declare_dram_parameter("woT", [2, 128, D], F16, isOutput=False)
    cosd = nc.declare_dram_parameter("cos2", [128, S], F16, isOutput=False)
    sind = nc.declare_dram_parameter("sinS2", [128, S], F16, isOutput=False)
    maskd = nc.declare_dram_parameter("bandmask", [128, WIN], F16, isOutput=False)
    permd = nc.declare_dram_parameter("permT", [128, 128], F16, isOutput=False)
    o_part = nc.declare_dram_parameter("o_part", [S, D], F16, isOutput=True)

    with tile.TileContext(nc) as tc:
        with (
            tc.tile_pool(name="persist", bufs=1) as pp,
            tc.tile_pool(name="work", bufs=6) as wkp,
            tc.tile_pool(name="ework", bufs=18) as ep,
            tc.tile_pool(name="norm", bufs=2) as npl,
            tc.tile_pool(name="ost", bufs=3) as opl,
        ):
            # ---- DMAs, ordered by first use; hsT streamed per dt tile ----
            wq_sb = pp.tile([128, 8, 256], F16, tag="wq")
            nc.sync.dma_start(out=wq_sb, in_=wq.rearrange("a p c -> p a c"))
            wkv_sb = pp.tile([128, 8, 128], F16, tag="wkv")
            nc.sync.dma_start(out=wkv_sb, in_=wkv.rearrange("a p c -> p a c"))
            perm_sb = pp.tile([128, 128], F16, tag="perm")
            nc.sync.dma_start(out=perm_sb, in_=permd[:, :])
            ht = []
            for dt in range(8):
                t = pp.tile([128, S], F16, tag=f"ht{dt}")
                nc.sync.dma_start(out=t, in_=hsT[dt][:, :])
                ht.append(t)
            cos_sb = pp.tile([128, S], F16, tag="cos")
            sin_sb = pp.tile([128, S], F16, tag="sin")
            nc.sync.dma_start(out=cos_sb, in_=cosd[:, :])
            nc.sync.dma_start(out=sin_sb, in_=sind[:, :])
            mask_sb = pp.tile([128, WIN], F16, tag="mask")
            nc.sync.dma_start(out=mask_sb, in_=maskd[:, :])
            wo_sb = pp.tile([128, 2, D], F16, tag="wo")
            nc.sync.dma_start(out=wo_sb, in_=wo.rearrange("a p c -> p a c"))

            # ---- on-chip init (gpsimd, overlaps DMA) ----
            ident = pp.tile([64, 64], F16, tag="ident")
            make_identity(nc, ident)
            warm = pp.tile([128, 512], F16, tag="warm")
            nc.gpsimd.memset(warm, 0.0)
            vload = []
            for kt in range(NKT):
                v = pp.tile([128, 192], F16, tag=f"vl{kt}")
                nc.gpsimd.memset(v, 1.0)
                vload.append(v)

            kdup = pp.tile([128, S], F16, tag="kdup")
            qT = [pp.tile([128, S], F16, tag=f"qT{m}", name=f"qT{m}") for m in range(2)]
            aT = [pp.tile([128, S], F16, tag=f"aT{ch}", name=f"aT{ch}") for ch in range(2)]

            # ---- phase 1: projections + rope ----
            with tc.tile_pool(name="p1", bufs=8, space="PSUM") as p1:
                wps = p1.tile([128, 512], F32, tag="p", name="warm")
                for _w in range(8):
                    nc.tensor.matmul(wps, warm[:, 0:128], warm,
                                     start=True, stop=True)

                qps = [[None] * NST for _ in range(2)]
                kvps = [None] * NST
                for dt in range(8):
                    for m in range(2):
                        for st in range(NST):
                            if dt == 0:
                                qps[m][st] = p1.tile([128, 512], F32, tag="p",
                                                     name=f"q{m}{st}")
                            nc.tensor.matmul(qps[m][st],
                                             wq_sb[:, dt, bass.ts(m, 128)],
                                             ht[dt][:, bass.ts(st, 512)],
                                             start=(dt == 0), stop=(dt == 7))
                for dt in range(8):
                    for st in range(NST):
                        if dt == 0:
                            kvps[st] = p1.tile([128, 512], F32, tag="p",
                                               name=f"kv{st}")
                        nc.tensor.matmul(kvps[st], wkv_sb[:, dt, :],
                                         ht[dt][:, bass.ts(st, 512)],
                                         start=(dt == 0), stop=(dt == 7))

                def rope_q(m):
                    for st in range(NST):
                        sl = bass.ts(st, 512)
                        raw = wkp.tile([128, 512], F16, tag="raw")
                        nc.scalar.copy(raw, qps[m][st])
                        rot = p1.tile([128, 512], F32, tag="p",
                                      name=f"rot{m}{st}")
                        nc.tensor.matmul(rot, perm_sb, raw, start=True, stop=True)
                        t1 = wkp.tile([128, 512], F16, tag="t1")
                        nc.vector.tensor_tensor(out=t1, in0=rot,
                                                in1=sin_sb[:, sl], op=OP.mult)
                        t2 = wkp.tile([128, 512], F16, tag="t2")
                        nc.gpsimd.tensor_tensor(out=t2, in0=raw,
                                                in1=cos_sb[:, sl], op=OP.mult)
                        nc.vector.tensor_tensor(out=qT[m][:, sl], in0=t1,
                                                in1=t2, op=OP.add)

                rope_q(0)
                rope_q(1)

                for st in range(NST):
                    sl = bass.ts(st, 512)
                    raw = wkp.tile([128, 512], F16, tag="rawkv")
                    nc.scalar.copy(raw, kvps[st])
                    rot = p1.tile([128, 512], F32, tag="p", name=f"rotkv{st}")
                    nc.tensor.matmul(rot[64:128, :], perm_sb[64:128, 64:128],
                                     raw[64:128, :], start=True, stop=True)
                    t1 = wkp.tile([128, 512], F16, tag="t1")
                    nc.vector.tensor_tensor(out=t1[64:128, :], in0=rot[64:128, :],
                                            in1=sin_sb[64:128, sl], op=OP.mult)
                    t2 = wkp.tile([128, 512], F16, tag="t2")
                    nc.gpsimd.tensor_tensor(out=t2[64:128, :], in0=raw[64:128, :],
                                            in1=cos_sb[64:128, sl], op=OP.mult)
                    nc.vector.tensor_tensor(out=kdup[64:128, sl],
                                            in0=t1[64:128, :], in1=t2[64:128, :],
                                            op=OP.add)
                    nc.sync.dma_start(out=kdup[0:64, sl], in_=kdup[64:128, sl])
                    for k4 in range(4):
                        kt = 4 * st + k4
                        tr = p1.tile([128, 64], F16, tag="p", name=f"tr{kt}")
                        nc.tensor.transpose(tr, raw[0:64, bass.ts(k4, 128)], ident)
                        nc.vector.tensor_copy(vload[kt][:, 64:128], tr)

            # ---- phases 2+3: banded attention + output projection ----
            with tc.tile_pool(name="pu", bufs=4, space="PSUM") as pu:
                expm = {}

                def emit_scores_kt(kt):
                    j0 = 128 * kt
                    w = min(WIN, S - j0)
                    for p in range(2):
                        sc = pu.tile([128, 2, 512], F32, tag="u",
                                     name=f"sc{kt}_{p}")
                        for hh in range(2):
                            h = 2 * p + hh
                            m, half = h // 2, (h % 2) * 64
                            nc.tensor.matmul(
                                sc[:, hh, 0:w],
                                kdup[half:half + 64, bass.ts(kt, 128)],
                                qT[m][half:half + 64, j0:j0 + w],
                                start=True, stop=True)
                        et2 = ep.tile([128, 2, WIN], F16, tag="e",
                                      name=f"e{kt}_{p}")
                        nc.scalar.activation(et2[:, :, 0:w], sc[:, :, 0:w],
                                             AF.Exp, scale=float(SCALE))
                        if w == WIN and MASK_BATCH:
                            blk = bass.AP(tensor=et2.tensor, offset=et2.offset,
                                          ap=[list(et2.ap[0]), [WIN, 2],
                                              [248, 2], [1, 136]])
                            mblk = bass.AP(tensor=mask_sb.tensor,
                                           offset=mask_sb.offset,
                                           ap=[list(mask_sb.ap[0]), [0, 2],
                                               [248, 2], [1, 136]])
                            nc.vector.tensor_tensor(out=blk, in0=blk, in1=mblk,
                                                    op=OP.mult)
                        else:
                            for hh in range(2):
                                nc.vector.tensor_tensor(
                                    out=et2[:, hh, 0:w], in0=et2[:, hh, 0:w],
                                    in1=mask_sb[:, 0:w], op=OP.mult)
                        expm[(kt, p)] = et2

                def emit_pv_pieces(pv, s, hp, pieces, started, close):
                    """Emit A/B piece matmuls; returns updated started flag."""
                    for i, (kt, c0, c1) in enumerate(pieces):
                        base = 128 * kt + c0 - 512 * s
                        last = close and (i == len(pieces) - 1)
                        et2 = expm[(kt, hp)]
                        nc.tensor.matmul(pv[:, 0, base:base + (c1 - c0)],
                                         vload[kt][:, 64:192],
                                         et2[:, 0, c0:c1],
                                         start=not started, stop=last)
                        nc.tensor.matmul(pv[:, 1, base:base + (c1 - c0)],
                                         vload[kt][:, 0:128],
                                         et2[:, 1, c0:c1],
                                         start=not started, stop=last)
                        started = True
                    return started

                def seed_pv(pv):
                    for c in range(2):
                        nc.tensor.matmul(pv[:, c, :], warm[:, 0:128], warm,
                                         start=True, stop=False)

                def emit_norm(s, hp, pv):
                    sl = bass.ts(s, 512)
                    rcp = npl.tile([128, 2, 512], F32, tag="rcp")
                    nc.vector.reciprocal_approx_fast(out=rcp[64:128, 0, :],
                                                     in_=pv[64:128, 0, :])
                    nc.vector.reciprocal_approx_fast(out=rcp[0:64, 1, :],
                                                     in_=pv[0:64, 1, :])
                    rcpS = npl.tile([128, 2, 512], F32, tag="rcpS")
                    nc.sync.dma_start(out=rcpS[0:64, 0, :], in_=rcp[64:128, 0, :])
                    nc.gpsimd.dma_start(out=rcpS[64:128, 1, :], in_=rcp[0:64, 1, :])
                    nc.vector.tensor_tensor(out=aT[hp][0:64, sl],
                                            in0=pv[0:64, 0, :],
                                            in1=rcpS[0:64, 0, :], op=OP.mult)
                    nc.vector.tensor_tensor(out=aT[hp][64:128, sl],
                                            in0=pv[64:128, 1, :],
                                            in1=rcpS[64:128, 1, :], op=OP.mult)

                def emit_pso(st):
                    psos = pu.tile([128, 2, 512], F32, tag="u", name=f"pso{st}")
                    for ch in range(2):
                        for d in range(2):
                            nc.tensor.matmul(psos[:, d, :],
                                             aT[ch][:, bass.ts(st, 128)],
                                             wo_sb[:, ch, bass.ts(d, 512)],
                                             start=(ch == 0), stop=(ch == 1))
                    ost = opl.tile([128, 1024], F16, tag="o")
                    nc.scalar.copy(ost[:, 0:512], psos[:, 0, :])
                    nc.vector.tensor_copy(ost[:, 512:1024], psos[:, 1, :])
                    nc.sync.dma_start(out=o_part[bass.ts(st, 128), :], in_=ost)

                for s in range(NST):
                    pieces = _pv_pieces(s)
                    oldp = [pc for pc in pieces if pc[0] < 4 * s]
                    newp = [pc for pc in pieces if pc[0] >= 4 * s]
                    fill = list(range(4 * (s - 1), 4 * s)) if s > 0 else []

                    pv0 = pu.tile([128, 2, 512], F32, tag="u", name=f"pv{s}0")
                    st0 = False
                    if PV_SEED:
                        seed_pv(pv0)
                        st0 = True
                    st0 = emit_pv_pieces(pv0, s, 0, oldp, st0, close=False)

                    for i, kt in enumerate(range(4 * s, 4 * s + 4)):
                        emit_scores_kt(kt)
                        if i >= 1 and fill:
                            emit_pso(fill[i - 1])

                    emit_pv_pieces(pv0, s, 0, newp, st0, close=True)
                    emit_norm(s, 0, pv0)

                    pv1 = pu.tile([128, 2, 512], F32, tag="u", name=f"pv{s}1")
                    st1 = False
                    if PV_SEED:
                        seed_pv(pv1)
                        st1 = True
                    emit_pv_pieces(pv1, s, 1, pieces, st1, close=True)
                    emit_norm(s, 1, pv1)
                    if fill:
                        emit_pso(fill[3])

                for st in range(4 * (NST - 1), 4 * NST):
                    emit_pso(st)

                if debug:
                    nc.sync.dma_start(out=dbg_qT0[:, :], in_=qT[0])
                    nc.sync.dma_start(out=dbg_kdup[:, :], in_=kdup)
                    nc.sync.dma_start(out=dbg_et[:, :, :], in_=expm[(5, 0)])
                    nc.sync.dma_start(out=dbg_aT0[:, :], in_=aT[0])

    nc.compile()
    return nc


_NC = {}


def _get_nc(debug=False):
    if debug not in _NC:
        _NC[debug] = _build(debug)
    return _NC[debug]


def _host_inputs(hidden_states, position_ids, Wq, Wk, Wv, Wo):
    hs = np.asarray(hidden_states, np.float32)
    Wq = np.asarray(Wq, np.float32)
    Wk = np.asarray(Wk, np.float32)
    Wv = np.asarray(Wv, np.float32)
    Wo = np.asarray(Wo, np.float32)

    hsT16 = [np.ascontiguousarray(hs[b].T).astype(np.float16).reshape(8, 128, S)
             for b in range(B)]

    inv_freq = (1.0 / (THETA ** (np.arange(0, HD, 2, dtype=np.float32) / HD))).astype(np.float32)
    cos2, sin2 = [], []
    for b in range(B):
        pos = np.asarray(position_ids[b]).astype(np.float32)
        freqs = pos[:, None] * inv_freq[None, :]          # [S, 32]
        cosf = np.cos(freqs).T                            # [32, S]
        sinf = np.sin(freqs).T
        cos64 = np.concatenate([cosf, cosf], axis=0)      # [64, S]
        sin64s = np.concatenate([-sinf, sinf], axis=0)    # sign-folded
        cos2.append(np.concatenate([cos64, cos64], axis=0).astype(np.float16))
        sin2.append(np.concatenate([sin64s, sin64s], axis=0).astype(np.float16))

    p = np.arange(128)[:, None]
    c = np.arange(WIN)[None, :]
    bandmask = ((p <= c) & (c < p + WINDOW)).astype(np.float16)

    perm = np.zeros((64, 64), np.float32)
    for i in range(32):
        perm[i, i + 32] = 1.0
        perm[i + 32, i] = 1.0
    perm2 = np.kron(np.eye(2, dtype=np.float32), perm)    # [128, 128]
    permT = np.ascontiguousarray(perm2.T).astype(np.float16)

    in_maps = []
    for core in range(8):
        b, g = divmod(core, 4)
        wqT = np.ascontiguousarray(Wq[256 * g:256 * (g + 1)].T).astype(np.float16).reshape(8, 128, 256)
        WKV = np.concatenate([Wv[64 * g:64 * (g + 1)], Wk[64 * g:64 * (g + 1)]], axis=0)  # [128, D]
        wkvT = np.ascontiguousarray(WKV.T).astype(np.float16).reshape(8, 128, 128)
        woT = np.ascontiguousarray(Wo[:, 256 * g:256 * (g + 1)].T).astype(np.float16).reshape(2, 128, D)
        in_maps.append({
            "hsT": hsT16[b], "wqT": wqT, "wkvT": wkvT, "woT": woT,
            "cos2": cos2[b], "sinS2": sin2[b],
            "bandmask": bandmask, "permT": permT,
        })
    return in_maps


def run_spmd(hidden_states, attention_mask, position_ids, Wq, Wk, Wv, Wo, debug=False, **spmd_kwargs):
    nc = _get_nc(debug)
    in_maps = _host_inputs(hidden_states, position_ids, Wq, Wk, Wv, Wo)
    res = run_bass_kernel_spmd(nc, in_maps, list(range(8)), **spmd_kwargs)
    out = np.zeros((B, S, D), np.float32)
    for core in range(8):
        out[core // 4] += np.asarray(res.results[core]["o_part"], np.float32)
    return out, res


def kernel(hidden_states, attention_mask, position_ids, Wq, Wk, Wv, Wo):
    out, _ = run_spmd(hidden_states, attention_mask, position_ids, Wq, Wk, Wv, Wo)
    return out


# revision 7
# speedup vs baseline: 1.6195x; 1.3342x over previous
"""Mimi-style GQA attention (RoPE + 250-wide sliding causal window) on 8 TRN2 NeuronCores.

Sharding (per spec hint): core c handles batch b=c//4 and KV-head group g=c%4
(4 query heads + 1 KV head). Wq/Wk/Wv column-sharded, Wo row-sharded along the
head dim; each core emits a partial [S, D] output (fp16); host sums the 4
partials per batch in fp32.

v2 design notes (vs the 174-200us baseline):
  * The baseline's softmax-normalization chain (1-lane copies, 3 SB-SB DMAs,
    gpsimd partition_broadcast per head pair) created >3.4us PE-idle windows,
    oscillating the HAM clock gate between 2.4 and 1.2 GHz for ~half the span.
  * PV stationaries are [V|ones] for even heads and [ones|V] for odd heads, so
    the softmax denominator lands broadcast across 64 PSUM partitions for free
    (matmul cost depends only on moving columns). Normalization is then two
    lane-aligned reciprocals, two small partition-shift DMAs, and two
    lane-aligned multiplies writing the head-stacked aT directly.
  * K+V projections fused into one [Wv|Wk] stationary (kT upper / vT lower
    partitions); RoPE for K runs entirely on partitions 64-127.
  * Projections stream dt-tiles of hsT as the DMA delivers them; 8 warm-up
    matmuls cover only the initial DMA latency.
  * PV accumulations use bank-aligned [128,2,512] PSUM tiles with start=True on
    the first banded piece (no zero-seed matmuls).
  * One shared 8-bank PSUM ring serves scores, PV, and out-projection tiles;
    out-projection matmuls of strip s-1 interleave between strip s's score
    bursts so the PE never waits on the normalization chain.
  * Output is written fp16 (halves output DMA bytes; host accumulates fp32).
"""
import os
import sys

for _p in ("/opt/trn_rl_repo", "/root/.axon_site/_ro/trn_rl_repo"):
    if os.path.isdir(_p) and _p not in sys.path:
        sys.path.append(_p)

import numpy as np
import concourse.bass as bass
import concourse.mybir as mybir
import concourse.tile as tile
from concourse import bacc
from concourse.bass_utils import run_bass_kernel_spmd
from concourse.masks import make_identity

F32 = mybir.dt.float32
F16 = mybir.dt.float16
AF = mybir.ActivationFunctionType
OP = mybir.AluOpType

B, S, D = 2, 2048, 1024
H, HK, HD = 16, 4, 64
WINDOW = 250
SCALE = 1.0 / np.sqrt(HD)
THETA = 10000.0
NKT = S // 128          # 16 k-tiles
NST = S // 512          # 4 q-strips
WIN = 384               # padded per-k-tile q-window

MASK_BATCH = True       # one strided mask multiply per head pair
PV_SEED = False         # fall back to zero-seed matmuls if bank clear misbehaves


def _pv_pieces(s):
    """PV pieces for q-strip s: list of (kt, c0, c1) window-column ranges."""
    out = []
    for kt in range(max(0, 4 * s - 2), min(NKT - 1, 4 * s + 3) + 1):
        j0 = 128 * kt
        w = min(WIN, S - j0)
        c_lo = max(0, 512 * s - j0)
        c_hi = min(w, 512 * (s + 1) - j0)
        if c_lo < c_hi:
            out.append((kt, c_lo, c_hi))
    return out


def _build(debug=False):
    nc = bacc.Bacc(None, target_bir_lowering=False)
    if debug:
        dbg_qT0 = nc.declare_dram_parameter("dbg_qT0", [128, S], F16, isOutput=True)
        dbg_kdup = nc.declare_dram_parameter("dbg_kdup", [128, S], F16, isOutput=True)
        dbg_et = nc.declare_dram_parameter("dbg_et", [128, 2, WIN], F16, isOutput=True)
        dbg_aT0 = nc.declare_dram_parameter("dbg_aT0", [128, S], F16, isOutput=True)

    hsT = nc.declare_dram_parameter("hsT", [8, 128, S], F16, isOutput=False)
    wq = nc.declare_dram_parameter("wqT", [8, 128, 256], F16, isOutput=False)
    wkv = nc.declare_dram_parameter("wkvT", [8, 128, 128], F16, isOutput=False)
    wo = nc.declare_dram_parameter("woT", [2, 128, D], F16, isOutput=False)
    cosd = nc.declare_dram_parameter("cos2", [128, S], F16, isOutput=False)
    sind = nc.declare_dram_parameter("sinS2", [128, S], F16, isOutput=False)
    maskd = nc.declare_dram_parameter("bandmask", [128, WIN], F16, isOutput=False)
    permd = nc.declare_dram_parameter("permT", [128, 128], F16, isOutput=False)
    o_part = nc.declare_dram_parameter("o_part", [S, D], F16, isOutput=True)

    with tile.TileContext(nc) as tc:
        with (
            tc.tile_pool(name="persist", bufs=1) as pp,
            tc.tile_pool(name="work", bufs=6) as wkp,
            tc.tile_pool(name="ework", bufs=10) as ep,
            tc.tile_pool(name="norm", bufs=2) as npl,
            tc.tile_pool(name="ost", bufs=3) as opl,
        ):
            # ---- DMAs: hsT streamed on the HWDGE/sync queue; weights and aux
            # tensors on the SWDGE/gpsimd queue so the two streams overlap ----
            warm = pp.tile([128, 512], F16, tag="warm")
            nc.gpsimd.memset(warm, 0.0)
            wq_sb = pp.tile([128, 8, 256], F16, tag="wq")
            nc.gpsimd.dma_start(out=wq_sb, in_=wq.rearrange("a p c -> p a c"))
            wkv_sb = pp.tile([128, 8, 128], F16, tag="wkv")
            nc.gpsimd.dma_start(out=wkv_sb, in_=wkv.rearrange("a p c -> p a c"))
            ht = []
            for dt in range(8):
                t = pp.tile([128, S], F16, tag=f"ht{dt}")
                nc.sync.dma_start(out=t, in_=hsT[dt][:, :])
                ht.append(t)
            ident = pp.tile([64, 64], F16, tag="ident")
            make_identity(nc, ident)
            vload = []
            for kt in range(NKT):
                v = pp.tile([128, 192], F16, tag=f"vl{kt}")
                nc.gpsimd.memset(v, 1.0)
                vload.append(v)
            perm_sb = pp.tile([128, 128], F16, tag="perm")
            nc.gpsimd.dma_start(out=perm_sb, in_=permd[:, :])
            cos_sb = pp.tile([128, S], F16, tag="cos")
            sin_sb = pp.tile([128, S], F16, tag="sin")
            nc.gpsimd.dma_start(out=cos_sb, in_=cosd[:, :])
            nc.gpsimd.dma_start(out=sin_sb, in_=sind[:, :])
            mask_sb = pp.tile([128, WIN], F16, tag="mask")
            nc.gpsimd.dma_start(out=mask_sb, in_=maskd[:, :])
            wo_sb = pp.tile([128, 2, D], F16, tag="wo")
            nc.gpsimd.dma_start(out=wo_sb, in_=wo.rearrange("a p c -> p a c"))

            kdup = pp.tile([128, S], F16, tag="kdup")
            qT = [pp.tile([128, S], F16, tag=f"qT{m}", name=f"qT{m}") for m in range(2)]
            aT = [pp.tile([128, S], F16, tag=f"aT{ch}", name=f"aT{ch}") for ch in range(2)]

            # ---- phase 1: projections + rope ----
            with tc.tile_pool(name="p1", bufs=8, space="PSUM") as p1:
                wps = p1.tile([128, 512], F32, tag="p", name="warm")
                for _w in range(14):
                    nc.tensor.matmul(wps, warm[:, 0:128], warm,
                                     start=True, stop=True)

                qps = [[None] * NST for _ in range(2)]
                kvps = [None] * NST

                def q_sweep(m):
                    for dt in range(8):
                        for st in range(NST):
                            if dt == 0:
                                qps[m][st] = p1.tile([128, 512], F32, tag="p",
                                                     name=f"q{m}{st}")
                            nc.tensor.matmul(qps[m][st],
                                             wq_sb[:, dt, bass.ts(m, 128)],
                                             ht[dt][:, bass.ts(st, 512)],
                                             start=(dt == 0), stop=(dt == 7))

                def kv_sweep():
                    for dt in range(8):
                        for st in range(NST):
                            if dt == 0:
                                kvps[st] = p1.tile([128, 512], F32, tag="p",
                                                   name=f"kv{st}")
                            nc.tensor.matmul(kvps[st], wkv_sb[:, dt, :],
                                             ht[dt][:, bass.ts(st, 512)],
                                             start=(dt == 0), stop=(dt == 7))

                def rope_q(m):
                    for st in range(NST):
                        sl = bass.ts(st, 512)
                        raw = wkp.tile([128, 512], F16, tag="raw")
                        nc.scalar.copy(raw, qps[m][st])
                        rot = p1.tile([128, 512], F32, tag="p",
                                      name=f"rot{m}{st}")
                        nc.tensor.matmul(rot, perm_sb, raw, start=True, stop=True)
                        t1 = wkp.tile([128, 512], F16, tag="t1")
                        nc.vector.tensor_tensor(out=t1, in0=rot,
                                                in1=sin_sb[:, sl], op=OP.mult)
                        t2 = wkp.tile([128, 512], F16, tag="t2")
                        nc.gpsimd.tensor_tensor(out=t2, in0=raw,
                                                in1=cos_sb[:, sl], op=OP.mult)
                        nc.vector.tensor_tensor(out=qT[m][:, sl], in0=t1,
                                                in1=t2, op=OP.add)

                def rope_kv(st):
                    sl = bass.ts(st, 512)
                    raw = wkp.tile([128, 512], F16, tag="rawkv")
                    rawkv.append(raw)
                    nc.scalar.copy(raw, kvps[st])
                    rot = p1.tile([128, 512], F32, tag="p", name=f"rotkv{st}")
                    nc.tensor.matmul(rot[64:128, :], perm_sb[64:128, 64:128],
                                     raw[64:128, :], start=True, stop=True)
                    t1 = wkp.tile([128, 512], F16, tag="t1")
                    nc.vector.tensor_tensor(out=t1[64:128, :], in0=rot[64:128, :],
                                            in1=sin_sb[64:128, sl], op=OP.mult)
                    t2 = wkp.tile([128, 512], F16, tag="t2")
                    nc.gpsimd.tensor_tensor(out=t2[64:128, :], in0=raw[64:128, :],
                                            in1=cos_sb[64:128, sl], op=OP.mult)
                    nc.vector.tensor_tensor(out=kdup[64:128, sl],
                                            in0=t1[64:128, :], in1=t2[64:128, :],
                                            op=OP.add)
                    nc.sync.dma_start(out=kdup[0:64, sl], in_=kdup[64:128, sl])

                rawkv = []
                q_sweep(0)
                kv_sweep()
                rope_q(0)
                for st in range(NST):
                    rope_kv(st)
                q_sweep(1)
                rope_q(1)
                # transposes grouped: only two PE transpose-mode switches
                for kt in range(NKT):
                    tr = p1.tile([128, 64], F16, tag="p", name=f"tr{kt}")
                    nc.tensor.transpose(tr, rawkv[kt // 4][0:64, bass.ts(kt % 4, 128)],
                                        ident)
                    nc.scalar.copy(vload[kt][:, 64:128], tr)

            # ---- phases 2+3: banded attention + output projection ----
            with tc.tile_pool(name="pu", bufs=4, space="PSUM") as pu:
                expm = {}

                def emit_scores_kt(kt):
                    j0 = 128 * kt
                    w = min(WIN, S - j0)
                    et4 = ep.tile([128, 4, WIN], F16, tag="e", name=f"e{kt}")
                    for p in range(2):
                        sc = pu.tile([128, 2, 512], F32, tag="u",
                                     name=f"sc{kt}_{p}")
                        for hh in range(2):
                            h = 2 * p + hh
                            m, half = h // 2, (h % 2) * 64
                            nc.tensor.matmul(
                                sc[:, hh, 0:w],
                                kdup[half:half + 64, bass.ts(kt, 128)],
                                qT[m][half:half + 64, j0:j0 + w],
                                start=True, stop=True)
                        nc.scalar.activation(et4[:, 2 * p:2 * p + 2, 0:w],
                                             sc[:, :, 0:w],
                                             AF.Exp, scale=float(SCALE))
                    if w == WIN and MASK_BATCH:
                        blk = bass.AP(tensor=et4.tensor, offset=et4.offset,
                                      ap=[list(et4.ap[0]), [WIN, 4],
                                          [248, 2], [1, 136]])
                        mblk = bass.AP(tensor=mask_sb.tensor,
                                       offset=mask_sb.offset,
                                       ap=[list(mask_sb.ap[0]), [0, 4],
                                           [248, 2], [1, 136]])
                        nc.vector.tensor_tensor(out=blk, in0=blk, in1=mblk,
                                                op=OP.mult)
                    else:
                        for h in range(4):
                            nc.vector.tensor_tensor(
                                out=et4[:, h, 0:w], in0=et4[:, h, 0:w],
                                in1=mask_sb[:, 0:w], op=OP.mult)
                    expm[kt] = et4

                def emit_pv_pieces(pv, s, hp, pieces, started, close):
                    """Emit A/B piece matmuls; returns updated started flag."""
                    for i, (kt, c0, c1) in enumerate(pieces):
                        base = 128 * kt + c0 - 512 * s
                        last = close and (i == len(pieces) - 1)
                        et4 = expm[kt]
                        nc.tensor.matmul(pv[:, 0, base:base + (c1 - c0)],
                                         vload[kt][:, 64:192],
                                         et4[:, 2 * hp, c0:c1],
                                         start=not started, stop=last)
                        nc.tensor.matmul(pv[:, 1, base:base + (c1 - c0)],
                                         vload[kt][:, 0:128],
                                         et4[:, 2 * hp + 1, c0:c1],
                                         start=not started, stop=last)
                        started = True
                    return started

                def seed_pv(pv):
                    for c in range(2):
                        nc.tensor.matmul(pv[:, c, :], warm[:, 0:128], warm,
                                         start=True, stop=False)

                def emit_norm(s, hp, pv):
                    sl = bass.ts(s, 512)
                    rcp = npl.tile([128, 2, 512], F32, tag="rcp")
                    # full-tile calls: reciprocal_approx_fast mislowers on
                    # partition-base-64 APs; unused lanes are don't-care
                    nc.vector.reciprocal_approx_fast(out=rcp[:, 0, :],
                                                     in_=pv[:, 0, :])
                    nc.vector.reciprocal_approx_fast(out=rcp[:, 1, :],
                                                     in_=pv[:, 1, :])
                    rcpS = npl.tile([128, 2, 512], F32, tag="rcpS")
                    nc.sync.dma_start(out=rcpS[0:64, 0, :], in_=rcp[64:128, 0, :])
                    nc.sync.dma_start(out=rcpS[64:128, 1, :], in_=rcp[0:64, 1, :])
                    nc.vector.tensor_tensor(out=aT[hp][0:64, sl],
                                            in0=pv[0:64, 0, :],
                                            in1=rcpS[0:64, 0, :], op=OP.mult)
                    nc.vector.tensor_tensor(out=aT[hp][64:128, sl],
                                            in0=pv[64:128, 1, :],
                                            in1=rcpS[64:128, 1, :], op=OP.mult)

                def emit_pso(st):
                    psos = pu.tile([128, 2, 512], F32, tag="u", name=f"pso{st}")
                    for ch in range(2):
                        for d in range(2):
                            nc.tensor.matmul(psos[:, d, :],
                                             aT[ch][:, bass.ts(st, 128)],
                                             wo_sb[:, ch, bass.ts(d, 512)],
                                             start=(ch == 0), stop=(ch == 1))
                    ost = opl.tile([128, 1024], F16, tag="o")
                    nc.scalar.copy(ost[:, 0:512], psos[:, 0, :])
                    nc.vector.tensor_copy(ost[:, 512:1024], psos[:, 1, :])
                    nc.sync.dma_start(out=o_part[bass.ts(st, 128), :], in_=ost)

                for s in range(NST):
                    pieces = _pv_pieces(s)
                    oldp = [pc for pc in pieces if pc[0] < 4 * s]
                    newp = [pc for pc in pieces if pc[0] >= 4 * s]
                    fill = list(range(4 * (s - 1), 4 * s)) if s > 0 else []

                    pv0 = pu.tile([128, 2, 512], F32, tag="u", name=f"pv{s}0")
                    st0 = False
                    if PV_SEED:
                        seed_pv(pv0)
                        st0 = True
                    st0 = emit_pv_pieces(pv0, s, 0, oldp, st0, close=False)

                    for i, kt in enumerate(range(4 * s, 4 * s + 4)):
                        emit_scores_kt(kt)
                        if i >= 1 and fill:
                            emit_pso(fill[i - 1])

                    emit_pv_pieces(pv0, s, 0, newp, st0, close=True)
                    emit_norm(s, 0, pv0)

                    pv1 = pu.tile([128, 2, 512], F32, tag="u", name=f"pv{s}1")
                    st1 = False
                    if PV_SEED:
                        seed_pv(pv1)
                        st1 = True
                    emit_pv_pieces(pv1, s, 1, pieces, st1, close=True)
                    emit_norm(s, 1, pv1)
                    if fill:
                        emit_pso(fill[3])

                for st in range(4 * (NST - 1), 4 * NST):
                    emit_pso(st)

                if debug:
                    nc.sync.dma_start(out=dbg_qT0[:, :], in_=qT[0])
                    nc.sync.dma_start(out=dbg_kdup[:, :], in_=kdup)
                    nc.sync.dma_start(out=dbg_et[:, :, :], in_=expm[5][:, 0:2, :])
                    nc.sync.dma_start(out=dbg_aT0[:, :], in_=aT[0])

    nc.compile()
    return nc


_NC = {}


def _get_nc(debug=False):
    if debug not in _NC:
        _NC[debug] = _build(debug)
    return _NC[debug]


def _host_inputs(hidden_states, position_ids, Wq, Wk, Wv, Wo):
    hs = np.asarray(hidden_states, np.float32)
    Wq = np.asarray(Wq, np.float32)
    Wk = np.asarray(Wk, np.float32)
    Wv = np.asarray(Wv, np.float32)
    Wo = np.asarray(Wo, np.float32)

    hsT16 = [np.ascontiguousarray(hs[b].T).astype(np.float16).reshape(8, 128, S)
             for b in range(B)]

    inv_freq = (1.0 / (THETA ** (np.arange(0, HD, 2, dtype=np.float32) / HD))).astype(np.float32)
    cos2, sin2 = [], []
    for b in range(B):
        pos = np.asarray(position_ids[b]).astype(np.float32)
        freqs = pos[:, None] * inv_freq[None, :]          # [S, 32]
        cosf = np.cos(freqs).T                            # [32, S]
        sinf = np.sin(freqs).T
        cos64 = np.concatenate([cosf, cosf], axis=0)      # [64, S]
        sin64s = np.concatenate([-sinf, sinf], axis=0)    # sign-folded
        cos2.append(np.concatenate([cos64, cos64], axis=0).astype(np.float16))
        sin2.append(np.concatenate([sin64s, sin64s], axis=0).astype(np.float16))

    p = np.arange(128)[:, None]
    c = np.arange(WIN)[None, :]
    bandmask = ((p <= c) & (c < p + WINDOW)).astype(np.float16)

    perm = np.zeros((64, 64), np.float32)
    for i in range(32):
        perm[i, i + 32] = 1.0
        perm[i + 32, i] = 1.0
    perm2 = np.kron(np.eye(2, dtype=np.float32), perm)    # [128, 128]
    permT = np.ascontiguousarray(perm2.T).astype(np.float16)

    in_maps = []
    for core in range(8):
        b, g = divmod(core, 4)
        wqT = np.ascontiguousarray(Wq[256 * g:256 * (g + 1)].T).astype(np.float16).reshape(8, 128, 256)
        WKV = np.concatenate([Wv[64 * g:64 * (g + 1)], Wk[64 * g:64 * (g + 1)]], axis=0)  # [128, D]
        wkvT = np.ascontiguousarray(WKV.T).astype(np.float16).reshape(8, 128, 128)
        woT = np.ascontiguousarray(Wo[:, 256 * g:256 * (g + 1)].T).astype(np.float16).reshape(2, 128, D)
        in_maps.append({
            "hsT": hsT16[b], "wqT": wqT, "wkvT": wkvT, "woT": woT,
            "cos2": cos2[b], "sinS2": sin2[b],
            "bandmask": bandmask, "permT": permT,
        })
    return in_maps


def run_spmd(hidden_states, attention_mask, position_ids, Wq, Wk, Wv, Wo, debug=False, **spmd_kwargs):
    nc = _get_nc(debug)
    in_maps = _host_inputs(hidden_states, position_ids, Wq, Wk, Wv, Wo)
    res = run_bass_kernel_spmd(nc, in_maps, list(range(8)), **spmd_kwargs)
    out = np.zeros((B, S, D), np.float32)
    for core in range(8):
        out[core // 4] += np.asarray(res.results[core]["o_part"], np.float32)
    return out, res


def kernel(hidden_states, attention_mask, position_ids, Wq, Wk, Wv, Wo):
    out, _ = run_spmd(hidden_states, attention_mask, position_ids, Wq, Wk, Wv, Wo)
    return out
